# revision 5
# baseline (speedup 1.0000x reference)
"""Trainium2 Bass kernel for a 2-layer GCN (nn_EvenLamerGCN) - supergather v4.

reference semantics (PyG GCNConv x2, eval mode):
    deg[i]  = 1 + indeg(i)                (self-loops added)
    dinv    = deg ** -0.5
    h  = relu(A_hat @ (x @ W1) + b1),  A_hat = D^-1/2 (A + I) D^-1/2
    o  = A_hat @ (h @ W2) + b2
    return o, log_softmax(o, axis=1)

Distribution: nodes sharded over 8 NeuronCores (12500/core, padded to
12544), edges partitioned by destination core.  The per-edge norm is
folded into per-node row scalings:
    out = dinv * ( sum_{e: dst=i} T[src_e] + T[i] ),   T = dinv * (x @ W)

Per layer on each core:
  1. dense matmul -> row-scaled table shard T_c; AllGather in PIECES so
     later pieces overlap upstream compute and the tail piece is small.
     The shard also stays resident in SBUF (self-loop rows need no
     reload).
  2. SWDGE supergathers: one dma_gather per (src window, group of OB dst
     blocks), round-robin over the 4 queues, fully static descriptor
     counts (padding reads row 0 and is discarded by the one-hot
     id == -1).  The per-queue descriptor rate (~8ns/desc) is the
     bottleneck, so the gather dispatch stream is decoupled from
     compute: index tiles load on the Scalar engine's DGE queue with
     2-group prefetch, and the first window's gathers pre-dispatch
     before the in-order AllGather instruction can block the queue.
  3. one PSUM group per 128-dst block: identity matmul (self-loop row)
     + one one-hot matmul per gathered chunk (+ rank-1 bias matmul only
     when biases are nonzero)
  4. epilogues run on the Scalar engine (activation with per-partition
     dinv scale); log_softmax's ln() is batched across all blocks to
     avoid Exp<->Ln activation-table thrashing.
Instruction streams are identical on all 8 cores (SPMD, one NEFF); all
per-core variation lives in input data.
"""

import sys

for _p in ("/opt/trn_rl_repo", "/root/.axon_site/_ro/trn_rl_repo"):
    if _p not in sys.path:
        sys.path.insert(0, _p)

from contextlib import ExitStack
from dataclasses import dataclass

import numpy as np

import concourse.bass as bass
import concourse.mybir as mybir
import concourse.tile as tile
from concourse import bacc
from concourse.bass import ds, ts
from concourse.bass_utils import run_bass_kernel_spmd
from concourse.masks import make_identity

F32 = mybir.dt.float32
BF16 = mybir.dt.bfloat16
I16 = mybir.dt.int16
I32 = mybir.dt.int32
AF = mybir.ActivationFunctionType
ALU = mybir.AluOpType

OB = 7                      # dst blocks per supergather group (98 = 14*7)
PRE = 3                     # window-0 gathers pre-dispatched per layer
PIECES1 = (49, 49)          # t1 AllGather pieces, in 128-row blocks
PIECES2 = (56, 32, 10)      # t2 AllGather pieces (small tail)


@dataclass(frozen=True)
class Cfg:
    n: int = 100000          # nodes
    din: int = 512           # input features
    dh: int = 128            # hidden features
    dout: int = 40           # output features
    cores: int = 8
    wsize: int = 32768       # int16 gather window (rows)
    max_piece: int = 32      # iota free-dim capacity (chunks)

    @property
    def nsh(self):           # real nodes per core
        return self.n // self.cores

    @property
    def nloc(self):          # padded nodes per core (multiple of 128)
        return ((self.nsh + 127) // 128) * 128

    @property
    def nt(self):            # 128-node dst blocks per core
        return self.nloc // 128

    @property
    def trows(self):         # rows in the gathered tables
        return self.cores * self.nloc

    @property
    def dh2(self):           # layer-2 compute/output width
        return self.dout

    @property
    def dt2(self):           # layer-2 bf16 table row width (256B rows)
        return max(128, self.dh2)

    @property
    def kt(self):            # k-tiles in the first matmul
        return self.din // 128

    @property
    def nwin(self):          # number of static src windows
        return max(1, -(-self.trows // self.wsize))

    @property
    def wbases(self):
        return [min(w * self.wsize, self.trows - self.wsize)
                for w in range(self.nwin)]


@dataclass(frozen=True)
class Plan:
    quotas1: tuple         # chunks per (window) cell, layer 1
    quotas2: tuple         # chunks per (window) cell, layer 2
    has_b1: bool
    has_b2: bool


# ----------------------------------------------------------------------------
# CPU-side preprocessing
# ----------------------------------------------------------------------------

def _piece_bounds(cfg, pieces):
    """local row bounds + global bases for a stacked-piece table layout."""
    lb = np.concatenate([[0], np.cumsum(np.array(pieces) * 128)])
    gb = lb * cfg.cores
    return lb, gb


def _layer_pack(cfg, r_src, b_all, id_all, core_all):
    """Build slot/ids arrays for one layer's table layout."""
    c = cfg
    w_all = np.minimum(r_src // c.wsize, c.nwin - 1)

    cell_key = (core_all * c.nt + b_all) * c.nwin + w_all
    counts = np.bincount(cell_key, minlength=c.cores * c.nt * c.nwin)
    counts = counts.reshape(c.cores, c.nt, c.nwin)
    quotas = tuple(int(-(-counts[:, :, w].max() // 128)) for w in range(c.nwin))

    bases = c.wbases
    cpb = sum(quotas)
    assert cpb <= c.max_piece
    offw = np.concatenate([[0], np.cumsum(quotas)])
    slots = c.nt * cpb * 128

    idx16 = np.zeros((c.cores, 128, slots // 16), np.int16)
    ids_f32 = np.empty((c.cores, 128, slots // 128), np.float32)

    order = np.lexsort((r_src, w_all, b_all, core_all))
    so_r, so_w, so_b, so_core, so_id = (
        r_src[order], w_all[order], b_all[order], core_all[order], id_all[order]
    )
    core_starts = np.searchsorted(so_core, np.arange(c.cores + 1))

    for ci in range(c.cores):
        lo, hi = core_starts[ci], core_starts[ci + 1]
        rr, ii = so_r[lo:hi], so_id[lo:hi]
        # slot layout (gather order): [og][w][bi][chunk]; pad = row 0
        rel = np.zeros(slots, np.int64)
        # ids layout (matmul chunk order): [b][w][chunk]; pad = -1
        ids = np.full(slots, -1.0, np.float32)
        pos = 0
        for b in range(c.nt):
            og, bi = divmod(b, OB)
            for w in range(c.nwin):
                cnt = counts[ci, b, w]
                if cnt:
                    soff = (og * OB * cpb + offw[w] * OB + bi * quotas[w]) * 128
                    rel[soff : soff + cnt] = rr[pos : pos + cnt] - bases[w]
                    ioff = (b * cpb + offw[w]) * 128
                    ids[ioff : ioff + cnt] = ii[pos : pos + cnt]
                    pos += cnt
        assert pos == hi - lo
        assert rel.max() < c.wsize and rel.min() >= 0

        v = rel.reshape(-1, 16)              # slot i at [i%16, i//16]
        wrapped = np.ascontiguousarray(v.T)  # [16, slots/16]
        idx16[ci] = np.tile(wrapped, (8, 1)).astype(np.int16)
        ids_f32[ci] = ids.reshape(slots // 128, 128).T

    return quotas, idx16, ids_f32


def preprocess(cfg: Cfg, edge_index: np.ndarray, b1, b2):
    c = cfg
    src = np.asarray(edge_index[0], dtype=np.int64)
    dst = np.asarray(edge_index[1], dtype=np.int64)

    deg = np.bincount(dst, minlength=c.n).astype(np.float32) + 1.0
    deg_pt = np.ones((c.cores, 128, c.nt), np.float32)
    sqd = np.ones((c.cores, 1, c.nloc), np.float32)
    for ci in range(c.cores):
        dl = np.ones(c.nloc, np.float32)
        dl[: c.nsh] = deg[ci * c.nsh : (ci + 1) * c.nsh]
        deg_pt[ci] = dl.reshape(c.nt, 128).T
        sqd[ci, 0] = np.sqrt(dl)

    def row_of(i, pieces):
        lb, gb = _piece_bounds(c, pieces)
        l = i % c.nsh
        cc = i // c.nsh
        p = np.searchsorted(lb, l, side="right") - 1
        psz = (lb[p + 1] - lb[p])
        return gb[p] + cc * psz + (l - lb[p])

    core_all = dst // c.nsh
    dloc_all = dst - core_all * c.nsh
    b_all = dloc_all // 128
    id_all = dloc_all % 128

    q1, idx16_1, ids_1 = _layer_pack(
        c, row_of(src, PIECES1), b_all, id_all, core_all)
    q2, idx16_2, ids_2 = _layer_pack(
        c, row_of(src, PIECES2), b_all, id_all, core_all)

    plan = Plan(quotas1=q1, quotas2=q2,
                has_b1=bool(np.any(np.asarray(b1))),
                has_b2=bool(np.any(np.asarray(b2))))
    return deg_pt, sqd, (idx16_1, ids_1), (idx16_2, ids_2), plan


# ----------------------------------------------------------------------------
# Device kernel
# ----------------------------------------------------------------------------

def build(nc, tc, cfg: Cfg, plan: Plan):
    c = cfg
    RG = [list(range(c.cores))]
    ng = c.nt // OB
    cpb1, cpb2 = sum(plan.quotas1), sum(plan.quotas2)
    any_bias = plan.has_b1 or plan.has_b2

    x_sh = nc.dram_tensor("x_sh", [c.din, c.nloc], BF16, kind="ExternalInput").ap()
    w1 = nc.dram_tensor("w1", [c.din, c.dh], BF16, kind="ExternalInput").ap()
    w2 = nc.dram_tensor("w2", [c.dh, c.dh2], BF16, kind="ExternalInput").ap()
    if plan.has_b1:
        b1r = nc.dram_tensor("b1r", [1, c.dh], BF16, kind="ExternalInput").ap()
    if plan.has_b2:
        b2r = nc.dram_tensor("b2r", [1, c.dh2], BF16, kind="ExternalInput").ap()
    degp = nc.dram_tensor("degp", [128, c.nt], F32, kind="ExternalInput").ap()
    if any_bias:
        sqdg = nc.dram_tensor("sqdg", [1, c.nloc], BF16, kind="ExternalInput").ap()
    idxd = [
        nc.dram_tensor(f"idx16_{i}", [128, c.nt * cpb * 8], I16,
                       kind="ExternalInput").ap()
        for i, cpb in ((1, cpb1), (2, cpb2))
    ]
    idsd = [
        nc.dram_tensor(f"idsf_{i}", [128, c.nt * cpb], BF16,
                       kind="ExternalInput").ap()
        for i, cpb in ((1, cpb1), (2, cpb2))
    ]
    out_h = nc.dram_tensor("out_h", [c.nloc, c.dh2], F32, kind="ExternalOutput").ap()
    out_ls = nc.dram_tensor("out_ls", [c.nloc, c.dh2], F32, kind="ExternalOutput").ap()

    t1_loc = nc.dram_tensor("t1_loc", [c.nloc, c.dh], BF16, kind="Internal").ap()
    t1_full = nc.dram_tensor(
        "t1_full", [c.trows, c.dh], BF16, kind="Internal", addr_space="Shared"
    ).ap()
    t2_loc = nc.dram_tensor("t2_loc", [c.nloc, c.dt2], BF16, kind="Internal").ap()
    t2_full = nc.dram_tensor(
        "t2_full", [c.trows, c.dt2], BF16, kind="Internal", addr_space="Shared"
    ).ap()

    self_gq = [0]

    def next_q():
        q = self_gq[0]
        self_gq[0] = (q + 1) % 4
        return q

    with ExitStack() as st:
        cpool = st.enter_context(tc.tile_pool(name="consts", bufs=1))
        gp = st.enter_context(tc.tile_pool(name="gp", bufs=3))
        sp = st.enter_context(tc.tile_pool(name="sp", bufs=3))
        pp = st.enter_context(tc.tile_pool(name="pp", bufs=3))
        ppsum = st.enter_context(tc.tile_pool(name="ppsum", bufs=3, space="PSUM"))
        p0 = st.enter_context(tc.tile_pool(name="p0", bufs=3))
        p0ps = st.enter_context(tc.tile_pool(name="p0ps", bufs=2, space="PSUM"))
        p0psT = st.enter_context(tc.tile_pool(name="p0psT", bufs=1, space="PSUM"))

        # ---- constants ----
        identb = cpool.tile([128, 128], BF16)
        make_identity(nc, identb)
        w1sb = cpool.tile([128, c.kt, c.dh], BF16)
        nc.sync.dma_start(w1sb, w1.rearrange("(o p) f -> p o f", p=128))
        w2sb = cpool.tile([128, c.dh2], BF16)
        nc.sync.dma_start(w2sb, w2)
        if plan.has_b1:
            b1sb = cpool.tile([1, c.dh], BF16)
            nc.sync.dma_start(b1sb, b1r)
        if plan.has_b2:
            b2sb = cpool.tile([1, c.dh2], BF16)
            nc.sync.dma_start(b2sb, b2r)
        if any_bias:
            sqsb = cpool.tile([1, c.nloc], BF16)
            nc.sync.dma_start(sqsb, sqdg)
        dinv = cpool.tile([128, c.nt], F32)
        nc.sync.dma_start(dinv, degp)
        nc.scalar.activation(dinv, dinv, AF.Sqrt)
        nc.vector.reciprocal(dinv, dinv)
        iota = cpool.tile([128, c.max_piece, 128], BF16)
        nc.gpsimd.iota(iota, pattern=[[0, c.max_piece], [1, 128]], base=0,
                       channel_multiplier=0,
                       allow_small_or_imprecise_dtypes=True)
        # SBUF-resident local table shards (self-loop rows)
        sdall1 = cpool.tile([128, c.nt, c.dh], BF16)
        sdall2 = cpool.tile([128, c.nt, c.dh2], BF16)
        # log_softmax deferred state
        sm_all = cpool.tile([128, c.nt, c.dh2], F32)
        se_all = cpool.tile([128, c.nt], F32)
        ln_all = cpool.tile([128, c.nt], F32)

        def gather_piece(t_loc, t_full, pieces, p):
            lb, gb = _piece_bounds(c, pieces)
            psz = int(lb[p + 1] - lb[p])
            nc.gpsimd.collective_compute(
                "AllGather", ALU.bypass, replica_groups=RG,
                ins=[t_loc[ds(int(lb[p]), psz), :].opt()],
                outs=[t_full[ds(int(gb[p]), c.cores * psz), :].opt()],
            )

        # ---- per-layer gather-phase state machine ----
        class LayerState:
            def __init__(self, quotas, idx16, idsf, table_full, dt):
                self.quotas = quotas
                self.cpb = sum(quotas)
                self.offw = [0]
                for qv in quotas:
                    self.offw.append(self.offw[-1] + qv)
                self.idx16 = idx16
                self.idsf = idsf
                self.table_full = table_full
                self.dt = dt
                self.sits = {}
                self.sids = {}
                self.glive = {}

            def load_idx(self, og):
                if og >= ng or og in self.sits:
                    return
                cpb = self.cpb
                sit = sp.tile([128, OB * cpb * 8], I16, tag="sit")
                nc.scalar.dma_start(
                    sit, self.idx16[:, og * OB * cpb * 8 : (og + 1) * OB * cpb * 8]
                )
                sid = sp.tile([128, OB * cpb], BF16, tag="sid")
                nc.scalar.dma_start(
                    sid, self.idsf[:, og * OB * cpb : (og + 1) * OB * cpb]
                )
                self.sits[og] = sit
                self.sids[og] = sid

            def dispatch(self, og, w):
                if (og, w) in self.glive:
                    return
                qv = self.quotas[w]
                g = gp.tile([128, OB * qv, self.dt], BF16, tag=f"gt{w}")
                nc.gpsimd.dma_gather(
                    g,
                    self.table_full[ds(c.wbases[w], c.wsize), :],
                    self.sits[og][:, self.offw[w] * OB * 8 : self.offw[w + 1] * OB * 8],
                    num_idxs=OB * qv * 128,
                    num_idxs_reg=OB * qv * 128,
                    elem_size=self.dt,
                    single_packet=False, queue_num=next_q(),
                )
                self.glive[(og, w)] = g

            def prelude(self):
                for og in range(min(PRE, ng)):
                    self.load_idx(og)
                    self.dispatch(og, 0)

        # ---- phase 0: T1 = dinv * (x @ W1), write local table shard ----
        ls1 = LayerState(plan.quotas1, idxd[0], idsd[0], t1_full, c.dh)
        ls2 = LayerState(plan.quotas2, idxd[1], idsd[1], t2_full, c.dt2)

        xv = x_sh.rearrange("(j p) n -> p j n", p=128)
        for t in range(c.nt):
            xt = p0.tile([128, c.kt, 128], BF16, tag="xt")
            nc.sync.dma_start(xt, xv[:, :, ts(t, 128)])
            hps = p0ps.tile([128, c.dh], F32, tag="hps")
            for j in range(c.kt):
                nc.tensor.matmul(
                    hps, lhsT=xt[:, j, :], rhs=w1sb[:, j, :],
                    start=(j == 0), stop=(j == c.kt - 1),
                )
            nc.scalar.activation(sdall1[:, t, :], hps, AF.Copy,
                                 scale=dinv[:, t : t + 1])
            nc.sync.dma_start(t1_loc[ts(t, 128), :], sdall1[:, t, :])
            if t == PIECES1[0] - 1:
                gather_piece(t1_loc, t1_full, PIECES1, 0)
        # window-0 gathers only need piece 0: dispatch them before the
        # second AllGather piece can block the gpsimd queue
        ls1.prelude()
        gather_piece(t1_loc, t1_full, PIECES1, 1)

        # ---- block-major edge aggregation ----
        def edge_phase(lstate, d, sdall, brow, post_block, hooks):
            quotas, cpb, offw = lstate.quotas, lstate.cpb, lstate.offw
            for og in range(ng):
                lstate.load_idx(og)       # no-op when prefetched
                lstate.load_idx(og + 1)
                lstate.load_idx(og + 2)
                for w in range(c.nwin):
                    lstate.dispatch(og, w)
                sid = lstate.sids[og]
                gts = [lstate.glive[(og, w)] for w in range(c.nwin)]
                for bi in range(OB):
                    b = og * OB + bi
                    stt = pp.tile([128, cpb, 128], BF16, tag="stt")
                    nc.vector.tensor_tensor(
                        stt, iota[:, :cpb, :],
                        sid[:, bi * cpb : (bi + 1) * cpb, None].to_broadcast(
                            (128, cpb, 128)
                        ),
                        ALU.is_equal,
                    )
                    ps = ppsum.tile([128, d], F32, tag="ps")
                    if brow is not None:
                        nc.tensor.matmul(ps, lhsT=sqsb[:, ts(b, 128)], rhs=brow,
                                         start=True, stop=False)
                    nc.tensor.matmul(ps, lhsT=identb, rhs=sdall[:, b, :],
                                     start=(brow is None), stop=False)
                    j = 0
                    for w, qv in enumerate(quotas):
                        for cc in range(qv):
                            j += 1
                            nc.tensor.matmul(
                                ps, lhsT=stt[:, offw[w] + cc, :],
                                rhs=gts[w][:, bi * qv + cc, :d],
                                start=False, stop=(j == cpb),
                            )
                    post_block(b, ps)
                    if b in hooks:
                        hooks[b]()
                for w in range(c.nwin):
                    del lstate.glive[(og, w)]

        # ---- layer 1: aggregate, then per block build the layer-2 table ----
        def post1(b, ps):
            g1 = p0.tile([128, c.dh], BF16, tag="g1")
            nc.scalar.activation(g1, ps, AF.Relu, scale=dinv[:, b : b + 1])
            g1b = p0.tile([128, c.dh], BF16, tag="g1b")
            nc.scalar.activation(g1b, g1, AF.Copy, scale=dinv[:, b : b + 1])
            tps = p0psT.tile([128, 128], BF16, tag="tps")
            nc.tensor.transpose(tps, g1b, identb)
            gT = p0.tile([128, 128], BF16, tag="gT")
            nc.vector.tensor_copy(gT, tps)
            h2ps = p0ps.tile([128, c.dh2], F32, tag="h2ps")
            nc.tensor.matmul(h2ps, lhsT=gT, rhs=w2sb, start=True, stop=True)
            nc.vector.tensor_copy(sdall2[:, b, :], h2ps)
            nc.sync.dma_start(t2_loc[ts(b, 128), : c.dh2], sdall2[:, b, :])

        # t2 AllGather pieces launch from inside the layer-1 block loop so
        # all but the small tail piece overlap the remaining layer-1 work
        pb = np.cumsum(PIECES2)
        hooks1 = {
            int(pb[0]) - 1: lambda: gather_piece(t2_loc, t2_full, PIECES2, 0),
            int(pb[1]) - 1: lambda: gather_piece(t2_loc, t2_full, PIECES2, 1),
        }
        edge_phase(ls1, c.dh, sdall1, b1sb if plan.has_b1 else None,
                   post1, hooks1)
        # layer-2 window-0 gathers only need piece 0: dispatch before the
        # tail AllGather piece blocks the gpsimd queue
        ls2.prelude()
        gather_piece(t2_loc, t2_full, PIECES2, 2)

        # ---- layer 2: aggregate, bias + softmax statistics per block ----
        ohv = out_h.rearrange("(t p) f -> p t f", p=128)
        olv = out_ls.rearrange("(t p) f -> p t f", p=128)

        def post2(b, ps):
            oh = p0.tile([128, c.dh2], F32, tag="oh")
            nc.scalar.activation(oh, ps, AF.Copy, scale=dinv[:, b : b + 1])
            nc.sync.dma_start(ohv[:, b, :], oh)
            mx = p0.tile([128, 1], F32, tag="mx")
            nc.vector.tensor_reduce(mx, oh, mybir.AxisListType.X, ALU.max)
            nc.vector.tensor_scalar_sub(sm_all[:, b, :], oh, mx)
            e1 = p0.tile([128, c.dh2], F32, tag="e1")
            nc.scalar.activation(e1, sm_all[:, b, :], AF.Exp,
                                 accum_out=se_all[:, b : b + 1])

        edge_phase(ls2, c.dh2, sdall2, b2sb if plan.has_b2 else None,
                   post2, {})

        # ---- deferred log_softmax epilogue: one Ln, one subtract, one DMA ----
        nc.scalar.activation(ln_all, se_all, AF.Ln)
        nc.vector.tensor_tensor(
            sm_all, sm_all,
            ln_all[:, :, None].to_broadcast((128, c.nt, c.dh2)),
            ALU.subtract,
        )
        nc.sync.dma_start(olv, sm_all)


# ----------------------------------------------------------------------------
# Host entry point
# ----------------------------------------------------------------------------

_CACHE = {}


def _get_compiled(cfg: Cfg, plan: Plan):
    key = (cfg, plan)
    if key not in _CACHE:
        nc = bacc.Bacc(
            "TRN2", target_bir_lowering=False, debug=False,
            num_devices=cfg.cores, num_swdge_queues=4,
        )
        with tile.TileContext(nc) as tc:
            build(nc, tc, cfg, plan)
        nc.compile()
        _CACHE[key] = nc
    return _CACHE[key]


def make_in_maps(cfg: Cfg, plan, x, W1, b1, W2, b2, deg_pt, sqd, pack1, pack2):
    import ml_dtypes

    c = cfg
    x = np.asarray(x, np.float32)
    w2p = np.asarray(W2, np.float32)[:, : c.dh2].astype(ml_dtypes.bfloat16)
    w1c = np.ascontiguousarray(
        np.asarray(W1, np.float32).astype(ml_dtypes.bfloat16)
    )
    idx16_1, ids_1 = pack1
    idx16_2, ids_2 = pack2

    in_maps = []
    for ci in range(c.cores):
        xs = np.zeros((c.din, c.nloc), ml_dtypes.bfloat16)
        xs[:, : c.nsh] = (
            x[ci * c.nsh : (ci + 1) * c.nsh].astype(ml_dtypes.bfloat16).T
        )
        m = {
            "x_sh": np.ascontiguousarray(xs),
            "w1": w1c,
            "w2": np.ascontiguousarray(w2p),
            "degp": np.ascontiguousarray(deg_pt[ci]),
            "idx16_1": np.ascontiguousarray(idx16_1[ci]),
            "idsf_1": np.ascontiguousarray(ids_1[ci].astype(ml_dtypes.bfloat16)),
            "idx16_2": np.ascontiguousarray(idx16_2[ci]),
            "idsf_2": np.ascontiguousarray(ids_2[ci].astype(ml_dtypes.bfloat16)),
        }
        if plan.has_b1:
            m["b1r"] = np.ascontiguousarray(
                np.asarray(b1, np.float32)[None, :].astype(ml_dtypes.bfloat16))
        if plan.has_b2:
            m["b2r"] = np.ascontiguousarray(
                np.asarray(b2, np.float32)[None, : c.dh2].astype(
                    ml_dtypes.bfloat16))
        if plan.has_b1 or plan.has_b2:
            m["sqdg"] = np.ascontiguousarray(sqd[ci].astype(ml_dtypes.bfloat16))
        in_maps.append(m)
    return in_maps


def _ensure_ntff_hook():
    """Install the axon NTFF profile hook if the image's antenv lacks it."""
    import types

    try:
        from antenv.axon_hooks import get_axon_ntff_profile_hook  # noqa: F401
        return
    except ImportError:
        pass
    import antenv

    m = types.ModuleType("antenv.axon_hooks")
    m._hook = None
    m.set_axon_ntff_profile_hook = lambda h: setattr(m, "_hook", h)
    m.get_axon_ntff_profile_hook = lambda: m._hook
    sys.modules["antenv.axon_hooks"] = m
    antenv.axon_hooks = m
    try:
        from trn_agent_boot.trn_boot import _ntff_profile_via_ctypes

        h = _ntff_profile_via_ctypes("/opt/axon/libaxon_pjrt.so")
        if h is not None:
            m._hook = h
    except Exception as e:
        print(f"ntff hook install failed: {e}")

    from concourse import bass_utils as bu

    bu.upload_artifacts = lambda tmpdir: tmpdir


def run(cfg: Cfg, inputs: dict, trace: bool = False):
    if trace:
        _ensure_ntff_hook()
    deg_pt, sqd, pack1, pack2, plan = preprocess(
        cfg, inputs["edge_index"], inputs["b1"], inputs["b2"])
    nc = _get_compiled(cfg, plan)
    in_maps = make_in_maps(
        cfg, plan, inputs["x"], inputs["W1"], inputs["b1"],
        inputs["W2"], inputs["b2"], deg_pt, sqd, pack1, pack2,
    )
    res = run_bass_kernel_spmd(
        nc, in_maps, core_ids=list(range(cfg.cores)), trace=trace
    )
    c = cfg
    h = np.concatenate(
        [res.results[ci]["out_h"][: c.nsh, : c.dout] for ci in range(c.cores)], axis=0
    )
    ls = np.concatenate(
        [res.results[ci]["out_ls"][: c.nsh, : c.dout] for ci in range(c.cores)], axis=0
    )
    return (h, ls), res


def kernel(**inputs):
    (h, ls), _ = run(Cfg(), inputs)
    return h, ls


# revision 8
# speedup vs baseline: 1.7122x; 1.7122x over previous
"""Trainium2 Bass kernel for a 2-layer GCN (nn_EvenLamerGCN) - block-major v5.

reference semantics (PyG GCNConv x2, eval mode):
    deg[i]  = 1 + indeg(i)                (self-loops added)
    dinv    = deg ** -0.5
    h  = relu(A_hat @ (x @ W1) + b1),  A_hat = D^-1/2 (A + I) D^-1/2
    o  = A_hat @ (h @ W2) + b2
    return o, log_softmax(o, axis=1)

Distribution: nodes sharded over 8 NeuronCores (12500/core, padded to
12544), edges partitioned by destination core.  The per-edge norm is
folded into per-node row scalings:
    out = dinv * ( sum_{e: dst=i} T[src_e] + T[i] ),   T = dinv * (x @ W)

Per layer on each core, processed PER 128-DST BLOCK (block-major):
  1. dense matmul -> row-scaled table shard T_c (kept SBUF-resident for
     the self-loop rows); AllGather in PIECES so later pieces overlap
     upstream compute and the tail piece is small.
  2. per block: one dma_gather per (block, src-window) cell with the
     cell's true edge count in num_idxs_reg (descriptor count == real
     edges), round-robin over the 4 SWDGE queues.  The per-queue
     descriptor-generation rate (~8ns/desc) is the bottleneck, so the
     gather stream is decoupled from compute: index/count tiles load on
     the Scalar engine's DGE queue and the gather buffer pool is deep
     (8 blocks in flight).
  3. one PSUM group per block: identity matmul adds the self-loop row,
     then one one-hot matmul per gathered chunk (+ rank-1 bias matmul
     only when biases are nonzero)
  4. epilogues run on the Scalar engine (activation with per-partition
     dinv scale); log_softmax's ln() is batched across all blocks to
     avoid Exp<->Ln activation-table thrashing.
Instruction streams are identical on all 8 cores (SPMD, one NEFF); all
per-core variation lives in input data (including per-cell counts read
into num_idxs_reg).
"""

import sys

for _p in ("/opt/trn_rl_repo", "/root/.axon_site/_ro/trn_rl_repo"):
    if _p not in sys.path:
        sys.path.insert(0, _p)

from contextlib import ExitStack
from dataclasses import dataclass

import numpy as np

import concourse.bass as bass
import concourse.mybir as mybir
import concourse.tile as tile
from concourse import bacc
from concourse.bass import ds, ts
from concourse.bass_utils import run_bass_kernel_spmd
from concourse.masks import make_identity

F32 = mybir.dt.float32
BF16 = mybir.dt.bfloat16
I16 = mybir.dt.int16
I32 = mybir.dt.int32
AF = mybir.ActivationFunctionType
ALU = mybir.AluOpType

OB = 7                      # dst blocks per index-load group (98 = 14*7)
PIECES1 = (49, 49)          # t1 AllGather pieces, in 128-row blocks
PIECES2 = (56, 32, 10)      # t2 AllGather pieces (small tail)


@dataclass(frozen=True)
class Cfg:
    n: int = 100000          # nodes
    din: int = 512           # input features
    dh: int = 128            # hidden features
    dout: int = 40           # output features
    cores: int = 8
    wsize: int = 32768       # int16 gather window (rows)
    max_piece: int = 32      # iota free-dim capacity (chunks)

    @property
    def nsh(self):           # real nodes per core
        return self.n // self.cores

    @property
    def nloc(self):          # padded nodes per core (multiple of 128)
        return ((self.nsh + 127) // 128) * 128

    @property
    def nt(self):            # 128-node dst blocks per core
        return self.nloc // 128

    @property
    def trows(self):         # rows in the gathered tables
        return self.cores * self.nloc

    @property
    def dh2(self):           # layer-2 compute/output width
        return self.dout

    @property
    def dt2(self):           # layer-2 bf16 table row width (256B rows)
        return max(128, self.dh2)

    @property
    def kt(self):            # k-tiles in the first matmul
        return self.din // 128

    @property
    def nwin(self):          # number of static src windows
        return max(1, -(-self.trows // self.wsize))

    @property
    def wbases(self):
        return [min(w * self.wsize, self.trows - self.wsize)
                for w in range(self.nwin)]


@dataclass(frozen=True)
class Plan:
    quotas1: tuple         # chunks per (window) cell, layer 1
    quotas2: tuple         # chunks per (window) cell, layer 2
    has_b1: bool
    has_b2: bool


# ----------------------------------------------------------------------------
# CPU-side preprocessing
# ----------------------------------------------------------------------------

def _piece_bounds(cfg, pieces):
    """local row bounds + global bases for a stacked-piece table layout."""
    lb = np.concatenate([[0], np.cumsum(np.array(pieces) * 128)])
    gb = lb * cfg.cores
    return lb, gb


def _layer_pack(cfg, r_src, b_all, id_all, core_all):
    """Build slot/ids/count arrays for one layer's table layout."""
    c = cfg
    w_all = np.minimum(r_src // c.wsize, c.nwin - 1)

    cell_key = (core_all * c.nt + b_all) * c.nwin + w_all
    counts = np.bincount(cell_key, minlength=c.cores * c.nt * c.nwin)
    counts = counts.reshape(c.cores, c.nt, c.nwin)
    quotas = tuple(int(-(-counts[:, :, w].max() // 128)) for w in range(c.nwin))

    bases = c.wbases
    cpb = sum(quotas)
    assert cpb <= c.max_piece
    offw = np.concatenate([[0], np.cumsum(quotas)])
    slots = c.nt * cpb * 128

    idx16 = np.zeros((c.cores, 128, slots // 16), np.int16)
    ids_f32 = np.empty((c.cores, 128, slots // 128), np.float32)
    cnts_wm = np.zeros((c.cores, c.nwin * c.nt), np.int32)

    order = np.lexsort((r_src, w_all, b_all, core_all))
    so_r, so_w, so_b, so_core, so_id = (
        r_src[order], w_all[order], b_all[order], core_all[order], id_all[order]
    )
    core_starts = np.searchsorted(so_core, np.arange(c.cores + 1))

    for ci in range(c.cores):
        lo, hi = core_starts[ci], core_starts[ci + 1]
        rr, ii = so_r[lo:hi], so_id[lo:hi]
        cnts_wm[ci] = counts[ci].reshape(-1)   # block-major [b][w]
        # slot layout: [b][w][chunk]; ids pad -1; trailing pad is trimmed
        # by num_idxs_reg (no descriptors generated for it)
        rel = np.full(slots, -1, np.int64)
        ids = np.full(slots, -1.0, np.float32)
        pos = 0
        for b in range(c.nt):
            for w in range(c.nwin):
                cnt = counts[ci, b, w]
                off = (b * cpb + offw[w]) * 128
                if cnt:
                    rel[off : off + cnt] = rr[pos : pos + cnt] - bases[w]
                    ids[off : off + cnt] = ii[pos : pos + cnt]
                    pos += cnt
        assert pos == hi - lo
        assert rel.max() < c.wsize

        v = rel.reshape(-1, 16)              # slot i at [i%16, i//16]
        wrapped = np.ascontiguousarray(v.T)  # [16, slots/16]
        idx16[ci] = np.tile(wrapped, (8, 1)).astype(np.int16)
        ids_f32[ci] = ids.reshape(slots // 128, 128).T

    return quotas, idx16, ids_f32, cnts_wm


def preprocess(cfg: Cfg, edge_index: np.ndarray, b1, b2):
    c = cfg
    src = np.asarray(edge_index[0], dtype=np.int64)
    dst = np.asarray(edge_index[1], dtype=np.int64)

    deg = np.bincount(dst, minlength=c.n).astype(np.float32) + 1.0
    deg_pt = np.ones((c.cores, 128, c.nt), np.float32)
    sqd = np.ones((c.cores, 1, c.nloc), np.float32)
    for ci in range(c.cores):
        dl = np.ones(c.nloc, np.float32)
        dl[: c.nsh] = deg[ci * c.nsh : (ci + 1) * c.nsh]
        deg_pt[ci] = dl.reshape(c.nt, 128).T
        sqd[ci, 0] = np.sqrt(dl)

    def row_of(i, pieces):
        lb, gb = _piece_bounds(c, pieces)
        l = i % c.nsh
        cc = i // c.nsh
        p = np.searchsorted(lb, l, side="right") - 1
        psz = (lb[p + 1] - lb[p])
        return gb[p] + cc * psz + (l - lb[p])

    core_all = dst // c.nsh
    dloc_all = dst - core_all * c.nsh
    b_all = dloc_all // 128
    id_all = dloc_all % 128

    pk1 = _layer_pack(c, row_of(src, PIECES1), b_all, id_all, core_all)
    pk2 = _layer_pack(c, row_of(src, PIECES2), b_all, id_all, core_all)

    plan = Plan(quotas1=pk1[0], quotas2=pk2[0],
                has_b1=bool(np.any(np.asarray(b1))),
                has_b2=bool(np.any(np.asarray(b2))))
    return deg_pt, sqd, pk1[1:], pk2[1:], plan


# ----------------------------------------------------------------------------
# Device kernel
# ----------------------------------------------------------------------------

def build(nc, tc, cfg: Cfg, plan: Plan):
    c = cfg
    RG = [list(range(c.cores))]
    cpb1, cpb2 = sum(plan.quotas1), sum(plan.quotas2)
    any_bias = plan.has_b1 or plan.has_b2

    x_sh = nc.dram_tensor("x_sh", [c.din, c.nloc], BF16, kind="ExternalInput").ap()
    w1 = nc.dram_tensor("w1", [c.din, c.dh], BF16, kind="ExternalInput").ap()
    w2 = nc.dram_tensor("w2", [c.dh, c.dh2], BF16, kind="ExternalInput").ap()
    if plan.has_b1:
        b1r = nc.dram_tensor("b1r", [1, c.dh], BF16, kind="ExternalInput").ap()
    if plan.has_b2:
        b2r = nc.dram_tensor("b2r", [1, c.dh2], BF16, kind="ExternalInput").ap()
    degp = nc.dram_tensor("degp", [128, c.nt], F32, kind="ExternalInput").ap()
    if any_bias:
        sqdg = nc.dram_tensor("sqdg", [1, c.nloc], BF16, kind="ExternalInput").ap()
    idxd, idsd, cntd = [], [], []
    for i, cpb in ((1, cpb1), (2, cpb2)):
        idxd.append(nc.dram_tensor(f"idx16_{i}", [128, c.nt * cpb * 8], I16,
                                   kind="ExternalInput").ap())
        idsd.append(nc.dram_tensor(f"idsf_{i}", [128, c.nt * cpb], BF16,
                                   kind="ExternalInput").ap())
        cntd.append(nc.dram_tensor(f"cnts32_{i}", [128, c.nwin * c.nt], I32,
                                   kind="ExternalInput").ap())
    out_h = nc.dram_tensor("out_h", [c.nloc, c.dh2], F32, kind="ExternalOutput").ap()
    out_ls = nc.dram_tensor("out_ls", [c.nloc, c.dh2], F32, kind="ExternalOutput").ap()

    t1_loc = nc.dram_tensor("t1_loc", [c.nloc, c.dh], BF16, kind="Internal").ap()
    t1_full = nc.dram_tensor(
        "t1_full", [c.trows, c.dh], BF16, kind="Internal", addr_space="Shared"
    ).ap()
    t2_loc = nc.dram_tensor("t2_loc", [c.nloc, c.dt2], BF16, kind="Internal").ap()
    t2_full = nc.dram_tensor(
        "t2_full", [c.trows, c.dt2], BF16, kind="Internal", addr_space="Shared"
    ).ap()

    self_gq = [0]

    def next_q():
        q = self_gq[0]
        self_gq[0] = (q + 1) % 4
        return q

    with ExitStack() as st:
        cpool = st.enter_context(tc.tile_pool(name="consts", bufs=1))
        gp = st.enter_context(tc.tile_pool(name="gp", bufs=8))
        sp = st.enter_context(tc.tile_pool(name="sp", bufs=3))
        pp = st.enter_context(tc.tile_pool(name="pp", bufs=4))
        ppsum = st.enter_context(tc.tile_pool(name="ppsum", bufs=3, space="PSUM"))
        p0 = st.enter_context(tc.tile_pool(name="p0", bufs=3))
        p0ps = st.enter_context(tc.tile_pool(name="p0ps", bufs=2, space="PSUM"))
        p0psT = st.enter_context(tc.tile_pool(name="p0psT", bufs=1, space="PSUM"))

        # ---- constants ----
        identb = cpool.tile([128, 128], BF16)
        make_identity(nc, identb)
        w1sb = cpool.tile([128, c.kt, c.dh], BF16)
        nc.sync.dma_start(w1sb, w1.rearrange("(o p) f -> p o f", p=128))
        w2sb = cpool.tile([128, c.dh2], BF16)
        nc.sync.dma_start(w2sb, w2)
        if plan.has_b1:
            b1sb = cpool.tile([1, c.dh], BF16)
            nc.sync.dma_start(b1sb, b1r)
        if plan.has_b2:
            b2sb = cpool.tile([1, c.dh2], BF16)
            nc.sync.dma_start(b2sb, b2r)
        if any_bias:
            sqsb = cpool.tile([1, c.nloc], BF16)
            nc.sync.dma_start(sqsb, sqdg)
        dinv = cpool.tile([128, c.nt], F32)
        nc.sync.dma_start(dinv, degp)
        nc.scalar.activation(dinv, dinv, AF.Sqrt)
        nc.vector.reciprocal(dinv, dinv)
        iota = cpool.tile([128, c.max_piece, 128], BF16)
        nc.gpsimd.iota(iota, pattern=[[0, c.max_piece], [1, 128]], base=0,
                       channel_multiplier=0,
                       allow_small_or_imprecise_dtypes=True)
        cntsb = [cpool.tile([128, c.nwin * c.nt], I32, name=f"cntsb{i}")
                 for i in range(2)]
        nc.scalar.dma_start(cntsb[0], cntd[0])
        nc.scalar.dma_start(cntsb[1], cntd[1])
        cregs = [
            nc.alloc_register(mybir.EngineType.Pool, f"gcnt{w}")
            for w in range(c.nwin)
        ]
        # SBUF-resident local table shards (self-loop rows)
        sdall1 = cpool.tile([128, c.nt, c.dh], BF16)
        sdall2 = cpool.tile([128, c.nt, c.dh2], BF16)
        # log_softmax deferred state
        sm_all = cpool.tile([128, c.nt, c.dh2], F32)
        se_all = cpool.tile([128, c.nt], F32)
        ln_all = cpool.tile([128, c.nt], F32)

        # zero the gather buffers once: reg-trimmed gathers leave padding
        # slots unwritten, and stale NaN-pattern garbage would poison the
        # 0*x one-hot matmul; afterwards stale data is old finite rows
        mp = max(cpb1, cpb2)
        for _ in range(8):
            gz = gp.tile([128, mp, 128], BF16, tag="gt")
            nc.vector.memset(gz, 0.0)

        def gather_piece(t_loc, t_full, pieces, p):
            lb, gb = _piece_bounds(c, pieces)
            psz = int(lb[p + 1] - lb[p])
            nc.gpsimd.collective_compute(
                "AllGather", ALU.bypass, replica_groups=RG,
                ins=[t_loc[ds(int(lb[p]), psz), :].opt()],
                outs=[t_full[ds(int(gb[p]), c.cores * psz), :].opt()],
            )

        # ---- phase 0: T1 = dinv * (x @ W1), write local table shard ----
        xv = x_sh.rearrange("(j p) n -> p j n", p=128)
        for t in range(c.nt):
            xt = p0.tile([128, c.kt, 128], BF16, tag="xt")
            nc.sync.dma_start(xt, xv[:, :, ts(t, 128)])
            hps = p0ps.tile([128, c.dh], F32, tag="hps")
            for j in range(c.kt):
                nc.tensor.matmul(
                    hps, lhsT=xt[:, j, :], rhs=w1sb[:, j, :],
                    start=(j == 0), stop=(j == c.kt - 1),
                )
            nc.scalar.activation(sdall1[:, t, :], hps, AF.Copy,
                                 scale=dinv[:, t : t + 1])
            nc.sync.dma_start(t1_loc[ts(t, 128), :], sdall1[:, t, :])
            if t == PIECES1[0] - 1:
                gather_piece(t1_loc, t1_full, PIECES1, 0)
        gather_piece(t1_loc, t1_full, PIECES1, 1)

        # ---- block-major edge aggregation ----
        def edge_phase(table_full, quotas, idx16, idsf, cnts, d, dt, sdall,
                       brow, post_block, hooks):
            cpb = sum(quotas)
            offw = [0]
            for qv in quotas:
                offw.append(offw[-1] + qv)
            for ob in range(0, c.nt, OB):
                sit = sp.tile([128, OB * cpb * 8], I16, tag="sit")
                nc.scalar.dma_start(
                    sit, idx16[:, ob * cpb * 8 : (ob + OB) * cpb * 8]
                )
                sid = sp.tile([128, OB * cpb], BF16, tag="sid")
                nc.scalar.dma_start(
                    sid, idsf[:, ob * cpb : (ob + OB) * cpb]
                )
                for bi in range(OB):
                    b = ob + bi
                    g = gp.tile([128, mp, 128], BF16, tag="gt")
                    nc.gpsimd.reg_load(
                        cregs, cnts[0:1, b * c.nwin : (b + 1) * c.nwin]
                    )
                    for w, qv in enumerate(quotas):
                        nc.gpsimd.dma_gather(
                            g[:, offw[w] : offw[w + 1], :],
                            table_full[ds(c.wbases[w], c.wsize), :],
                            sit[:, (bi * cpb + offw[w]) * 8
                                : (bi * cpb + offw[w + 1]) * 8],
                            num_idxs=qv * 128, num_idxs_reg=cregs[w],
                            elem_size=dt,
                            single_packet=False, queue_num=next_q(),
                        )
                    stt = pp.tile([128, cpb, 128], BF16, tag="stt")
                    nc.vector.tensor_tensor(
                        stt, iota[:, :cpb, :],
                        sid[:, bi * cpb : (bi + 1) * cpb, None].to_broadcast(
                            (128, cpb, 128)
                        ),
                        ALU.is_equal,
                    )
                    ps = ppsum.tile([128, d], F32, tag="ps")
                    if brow is not None:
                        nc.tensor.matmul(ps, lhsT=sqsb[:, ts(b, 128)],
                                         rhs=brow, start=True, stop=False)
                    nc.tensor.matmul(ps, lhsT=identb, rhs=sdall[:, b, :],
                                     start=(brow is None), stop=False)
                    for j in range(cpb):
                        nc.tensor.matmul(
                            ps, lhsT=stt[:, j, :], rhs=g[:, j, :d],
                            start=False, stop=(j == cpb - 1),
                        )
                    post_block(b, ps)
                    if b in hooks:
                        hooks[b]()

        # ---- layer 1: aggregate, then per block build the layer-2 table ----
        def post1(b, ps):
            g1 = p0.tile([128, c.dh], BF16, tag="g1")
            nc.scalar.activation(g1, ps, AF.Relu, scale=dinv[:, b : b + 1])
            g1b = p0.tile([128, c.dh], BF16, tag="g1b")
            nc.scalar.activation(g1b, g1, AF.Copy, scale=dinv[:, b : b + 1])
            tps = p0psT.tile([128, 128], BF16, tag="tps")
            nc.tensor.transpose(tps, g1b, identb)
            gT = p0.tile([128, 128], BF16, tag="gT")
            nc.vector.tensor_copy(gT, tps)
            h2ps = p0ps.tile([128, c.dh2], F32, tag="h2ps")
            nc.tensor.matmul(h2ps, lhsT=gT, rhs=w2sb, start=True, stop=True)
            nc.vector.tensor_copy(sdall2[:, b, :], h2ps)
            nc.sync.dma_start(t2_loc[ts(b, 128), : c.dh2], sdall2[:, b, :])

        # t2 AllGather pieces launch from inside the layer-1 block loop so
        # all but the small tail piece overlap the remaining layer-1 work
        pb = np.cumsum(PIECES2)
        hooks1 = {
            int(pb[0]) - 1: lambda: gather_piece(t2_loc, t2_full, PIECES2, 0),
            int(pb[1]) - 1: lambda: gather_piece(t2_loc, t2_full, PIECES2, 1),
        }
        edge_phase(t1_full, plan.quotas1, idxd[0], idsd[0], cntsb[0],
                   c.dh, c.dh, sdall1, b1sb if plan.has_b1 else None,
                   post1, hooks1)
        gather_piece(t2_loc, t2_full, PIECES2, 2)

        # ---- layer 2: aggregate, bias + softmax statistics per block ----
        ohv = out_h.rearrange("(t p) f -> p t f", p=128)
        olv = out_ls.rearrange("(t p) f -> p t f", p=128)

        def post2(b, ps):
            oh = p0.tile([128, c.dh2], F32, tag="oh")
            nc.scalar.activation(oh, ps, AF.Copy, scale=dinv[:, b : b + 1])
            nc.sync.dma_start(ohv[:, b, :], oh)
            mx = p0.tile([128, 1], F32, tag="mx")
            nc.vector.tensor_reduce(mx, oh, mybir.AxisListType.X, ALU.max)
            nc.vector.tensor_scalar_sub(sm_all[:, b, :], oh, mx)
            e1 = p0.tile([128, c.dh2], F32, tag="e1")
            nc.scalar.activation(e1, sm_all[:, b, :], AF.Exp,
                                 accum_out=se_all[:, b : b + 1])

        edge_phase(t2_full, plan.quotas2, idxd[1], idsd[1], cntsb[1],
                   c.dh2, c.dt2, sdall2, b2sb if plan.has_b2 else None,
                   post2, {})

        # ---- deferred log_softmax epilogue: one Ln, one subtract, one DMA ----
        nc.scalar.activation(ln_all, se_all, AF.Ln)
        nc.vector.tensor_tensor(
            sm_all, sm_all,
            ln_all[:, :, None].to_broadcast((128, c.nt, c.dh2)),
            ALU.subtract,
        )
        nc.sync.dma_start(olv, sm_all)


# ----------------------------------------------------------------------------
# Host entry point
# ----------------------------------------------------------------------------

_CACHE = {}


def _get_compiled(cfg: Cfg, plan: Plan):
    key = (cfg, plan)
    if key not in _CACHE:
        nc = bacc.Bacc(
            "TRN2", target_bir_lowering=False, debug=False,
            num_devices=cfg.cores, num_swdge_queues=4,
        )
        with tile.TileContext(nc) as tc:
            build(nc, tc, cfg, plan)
        nc.compile()
        _CACHE[key] = nc
    return _CACHE[key]


def make_in_maps(cfg: Cfg, plan, x, W1, b1, W2, b2, deg_pt, sqd, pack1, pack2):
    import ml_dtypes

    c = cfg
    x = np.asarray(x, np.float32)
    w2p = np.asarray(W2, np.float32)[:, : c.dh2].astype(ml_dtypes.bfloat16)
    w1c = np.ascontiguousarray(
        np.asarray(W1, np.float32).astype(ml_dtypes.bfloat16)
    )
    idx16_1, ids_1, cnts_1 = pack1
    idx16_2, ids_2, cnts_2 = pack2

    in_maps = []
    for ci in range(c.cores):
        xs = np.zeros((c.din, c.nloc), ml_dtypes.bfloat16)
        xs[:, : c.nsh] = (
            x[ci * c.nsh : (ci + 1) * c.nsh].astype(ml_dtypes.bfloat16).T
        )
        m = {
            "x_sh": np.ascontiguousarray(xs),
            "w1": w1c,
            "w2": np.ascontiguousarray(w2p),
            "degp": np.ascontiguousarray(deg_pt[ci]),
            "idx16_1": np.ascontiguousarray(idx16_1[ci]),
            "idsf_1": np.ascontiguousarray(ids_1[ci].astype(ml_dtypes.bfloat16)),
            "cnts32_1": np.ascontiguousarray(
                np.tile(cnts_1[ci][None, :], (128, 1))),
            "idx16_2": np.ascontiguousarray(idx16_2[ci]),
            "idsf_2": np.ascontiguousarray(ids_2[ci].astype(ml_dtypes.bfloat16)),
            "cnts32_2": np.ascontiguousarray(
                np.tile(cnts_2[ci][None, :], (128, 1))),
        }
        if plan.has_b1:
            m["b1r"] = np.ascontiguousarray(
                np.asarray(b1, np.float32)[None, :].astype(ml_dtypes.bfloat16))
        if plan.has_b2:
            m["b2r"] = np.ascontiguousarray(
                np.asarray(b2, np.float32)[None, : c.dh2].astype(
                    ml_dtypes.bfloat16))
        if plan.has_b1 or plan.has_b2:
            m["sqdg"] = np.ascontiguousarray(sqd[ci].astype(ml_dtypes.bfloat16))
        in_maps.append(m)
    return in_maps


def _ensure_ntff_hook():
    """Install the axon NTFF profile hook if the image's antenv lacks it."""
    import types

    try:
        from antenv.axon_hooks import get_axon_ntff_profile_hook  # noqa: F401
        return
    except ImportError:
        pass
    import antenv

    m = types.ModuleType("antenv.axon_hooks")
    m._hook = None
    m.set_axon_ntff_profile_hook = lambda h: setattr(m, "_hook", h)
    m.get_axon_ntff_profile_hook = lambda: m._hook
    sys.modules["antenv.axon_hooks"] = m
    antenv.axon_hooks = m
    try:
        from trn_agent_boot.trn_boot import _ntff_profile_via_ctypes

        h = _ntff_profile_via_ctypes("/opt/axon/libaxon_pjrt.so")
        if h is not None:
            m._hook = h
    except Exception as e:
        print(f"ntff hook install failed: {e}")

    from concourse import bass_utils as bu

    bu.upload_artifacts = lambda tmpdir: tmpdir


def run(cfg: Cfg, inputs: dict, trace: bool = False):
    if trace:
        _ensure_ntff_hook()
    deg_pt, sqd, pack1, pack2, plan = preprocess(
        cfg, inputs["edge_index"], inputs["b1"], inputs["b2"])
    nc = _get_compiled(cfg, plan)
    in_maps = make_in_maps(
        cfg, plan, inputs["x"], inputs["W1"], inputs["b1"],
        inputs["W2"], inputs["b2"], deg_pt, sqd, pack1, pack2,
    )
    res = run_bass_kernel_spmd(
        nc, in_maps, core_ids=list(range(cfg.cores)), trace=trace
    )
    c = cfg
    h = np.concatenate(
        [res.results[ci]["out_h"][: c.nsh, : c.dout] for ci in range(c.cores)], axis=0
    )
    ls = np.concatenate(
        [res.results[ci]["out_ls"][: c.nsh, : c.dout] for ci in range(c.cores)], axis=0
    )
    return (h, ls), res


def kernel(**inputs):
    (h, ls), _ = run(Cfg(), inputs)
    return h, ls


# revision 11
# speedup vs baseline: 1.7180x; 1.0034x over previous
"""Trainium2 Bass kernel for a 2-layer GCN (nn_EvenLamerGCN) - block-major v5.

reference semantics (PyG GCNConv x2, eval mode):
    deg[i]  = 1 + indeg(i)                (self-loops added)
    dinv    = deg ** -0.5
    h  = relu(A_hat @ (x @ W1) + b1),  A_hat = D^-1/2 (A + I) D^-1/2
    o  = A_hat @ (h @ W2) + b2
    return o, log_softmax(o, axis=1)

Distribution: nodes sharded over 8 NeuronCores (12500/core, padded to
12544), edges partitioned by destination core.  The per-edge norm is
folded into per-node row scalings:
    out = dinv * ( sum_{e: dst=i} T[src_e] + T[i] ),   T = dinv * (x @ W)

Per layer on each core, processed PER 128-DST BLOCK (block-major):
  1. dense matmul -> row-scaled table shard T_c (kept SBUF-resident for
     the self-loop rows); AllGather in PIECES so later pieces overlap
     upstream compute and the tail piece is small.
  2. per block: one dma_gather per (block, src-window) cell with the
     cell's true edge count in num_idxs_reg (descriptor count == real
     edges), round-robin over the 4 SWDGE queues.  The per-queue
     descriptor-generation rate (~8ns/desc) is the bottleneck, so the
     gather stream is decoupled from compute: index/count tiles load on
     the Scalar engine's DGE queue and the gather buffer pool is deep
     (8 blocks in flight).
  3. one PSUM group per block: identity matmul adds the self-loop row,
     then one one-hot matmul per gathered chunk (+ rank-1 bias matmul
     only when biases are nonzero)
  4. epilogues run on the Scalar engine (activation with per-partition
     dinv scale); log_softmax's ln() is batched across all blocks to
     avoid Exp<->Ln activation-table thrashing.
Instruction streams are identical on all 8 cores (SPMD, one NEFF); all
per-core variation lives in input data (including per-cell counts read
into num_idxs_reg).
"""

import sys

for _p in ("/opt/trn_rl_repo", "/root/.axon_site/_ro/trn_rl_repo"):
    if _p not in sys.path:
        sys.path.insert(0, _p)

from contextlib import ExitStack
from dataclasses import dataclass

import numpy as np

import concourse.bass as bass
import concourse.mybir as mybir
import concourse.tile as tile
from concourse import bacc
from concourse.bass import ds, ts
from concourse.bass_utils import run_bass_kernel_spmd
from concourse.masks import make_identity

F32 = mybir.dt.float32
BF16 = mybir.dt.bfloat16
I16 = mybir.dt.int16
I32 = mybir.dt.int32
AF = mybir.ActivationFunctionType
ALU = mybir.AluOpType

OB = 7                      # dst blocks per index-load group (98 = 14*7)
PIECES1 = (49, 49)          # t1 AllGather pieces, in 128-row blocks
PIECES2 = (56, 32, 10)      # t2 AllGather pieces (small tail)


@dataclass(frozen=True)
class Cfg:
    n: int = 100000          # nodes
    din: int = 512           # input features
    dh: int = 128            # hidden features
    dout: int = 40           # output features
    cores: int = 8
    wsize: int = 32768       # int16 gather window (rows)
    max_piece: int = 32      # iota free-dim capacity (chunks)

    @property
    def nsh(self):           # real nodes per core
        return self.n // self.cores

    @property
    def nloc(self):          # padded nodes per core (multiple of 128)
        return ((self.nsh + 127) // 128) * 128

    @property
    def nt(self):            # 128-node dst blocks per core
        return self.nloc // 128

    @property
    def trows(self):         # rows in the gathered tables
        return self.cores * self.nloc

    @property
    def dh2(self):           # layer-2 compute/output width
        return self.dout

    @property
    def dt2(self):           # layer-2 bf16 table row width (256B rows)
        return max(128, self.dh2)

    @property
    def kt(self):            # k-tiles in the first matmul
        return self.din // 128

    @property
    def nwin(self):          # number of static src windows
        return max(1, -(-self.trows // self.wsize))

    @property
    def wbases(self):
        return [min(w * self.wsize, self.trows - self.wsize)
                for w in range(self.nwin)]


@dataclass(frozen=True)
class Plan:
    quotas1: tuple         # chunks per (window) cell, layer 1
    quotas2: tuple         # chunks per (window) cell, layer 2
    has_b1: bool
    has_b2: bool


# ----------------------------------------------------------------------------
# CPU-side preprocessing
# ----------------------------------------------------------------------------

def _piece_bounds(cfg, pieces):
    """local row bounds + global bases for a stacked-piece table layout."""
    lb = np.concatenate([[0], np.cumsum(np.array(pieces) * 128)])
    gb = lb * cfg.cores
    return lb, gb


def _layer_pack(cfg, r_src, b_all, id_all, core_all):
    """Build slot/ids/count arrays for one layer's table layout."""
    c = cfg
    w_all = np.minimum(r_src // c.wsize, c.nwin - 1)

    cell_key = (core_all * c.nt + b_all) * c.nwin + w_all
    counts = np.bincount(cell_key, minlength=c.cores * c.nt * c.nwin)
    counts = counts.reshape(c.cores, c.nt, c.nwin)
    quotas = tuple(int(-(-counts[:, :, w].max() // 128)) for w in range(c.nwin))

    bases = c.wbases
    cpb = sum(quotas)
    assert cpb <= c.max_piece
    offw = np.concatenate([[0], np.cumsum(quotas)])
    slots = c.nt * cpb * 128

    idx16 = np.zeros((c.cores, 128, slots // 16), np.int16)
    ids_f32 = np.empty((c.cores, 128, slots // 128), np.float32)
    cnts_wm = np.zeros((c.cores, c.nwin * c.nt), np.int32)

    order = np.lexsort((r_src, w_all, b_all, core_all))
    so_r, so_w, so_b, so_core, so_id = (
        r_src[order], w_all[order], b_all[order], core_all[order], id_all[order]
    )
    core_starts = np.searchsorted(so_core, np.arange(c.cores + 1))

    for ci in range(c.cores):
        lo, hi = core_starts[ci], core_starts[ci + 1]
        rr, ii = so_r[lo:hi], so_id[lo:hi]
        cnts_wm[ci] = counts[ci].reshape(-1)   # block-major [b][w]
        # slot layout: [b][w][chunk]; ids pad -1; trailing pad is trimmed
        # by num_idxs_reg (no descriptors generated for it)
        rel = np.full(slots, -1, np.int64)
        ids = np.full(slots, -1.0, np.float32)
        pos = 0
        for b in range(c.nt):
            for w in range(c.nwin):
                cnt = counts[ci, b, w]
                off = (b * cpb + offw[w]) * 128
                if cnt:
                    rel[off : off + cnt] = rr[pos : pos + cnt] - bases[w]
                    ids[off : off + cnt] = ii[pos : pos + cnt]
                    pos += cnt
        assert pos == hi - lo
        assert rel.max() < c.wsize

        v = rel.reshape(-1, 16)              # slot i at [i%16, i//16]
        wrapped = np.ascontiguousarray(v.T)  # [16, slots/16]
        idx16[ci] = np.tile(wrapped, (8, 1)).astype(np.int16)
        ids_f32[ci] = ids.reshape(slots // 128, 128).T

    return quotas, idx16, ids_f32, cnts_wm


def preprocess(cfg: Cfg, edge_index: np.ndarray, b1, b2):
    c = cfg
    src = np.asarray(edge_index[0], dtype=np.int64)
    dst = np.asarray(edge_index[1], dtype=np.int64)

    deg = np.bincount(dst, minlength=c.n).astype(np.float32) + 1.0
    deg_pt = np.ones((c.cores, 128, c.nt), np.float32)
    sqd = np.ones((c.cores, 1, c.nloc), np.float32)
    for ci in range(c.cores):
        dl = np.ones(c.nloc, np.float32)
        dl[: c.nsh] = deg[ci * c.nsh : (ci + 1) * c.nsh]
        deg_pt[ci] = dl.reshape(c.nt, 128).T
        sqd[ci, 0] = np.sqrt(dl)

    def row_of(i, pieces):
        lb, gb = _piece_bounds(c, pieces)
        l = i % c.nsh
        cc = i // c.nsh
        p = np.searchsorted(lb, l, side="right") - 1
        psz = (lb[p + 1] - lb[p])
        return gb[p] + cc * psz + (l - lb[p])

    core_all = dst // c.nsh
    dloc_all = dst - core_all * c.nsh
    b_all = dloc_all // 128
    id_all = dloc_all % 128

    pk1 = _layer_pack(c, row_of(src, PIECES1), b_all, id_all, core_all)
    pk2 = _layer_pack(c, row_of(src, PIECES2), b_all, id_all, core_all)

    plan = Plan(quotas1=pk1[0], quotas2=pk2[0],
                has_b1=bool(np.any(np.asarray(b1))),
                has_b2=bool(np.any(np.asarray(b2))))
    return deg_pt, sqd, pk1[1:], pk2[1:], plan


# ----------------------------------------------------------------------------
# Device kernel
# ----------------------------------------------------------------------------

def build(nc, tc, cfg: Cfg, plan: Plan):
    c = cfg
    RG = [list(range(c.cores))]
    cpb1, cpb2 = sum(plan.quotas1), sum(plan.quotas2)
    any_bias = plan.has_b1 or plan.has_b2

    x_sh = nc.dram_tensor("x_sh", [c.din, c.nloc], BF16, kind="ExternalInput").ap()
    w1 = nc.dram_tensor("w1", [c.din, c.dh], BF16, kind="ExternalInput").ap()
    w2 = nc.dram_tensor("w2", [c.dh, c.dh2], BF16, kind="ExternalInput").ap()
    if plan.has_b1:
        b1r = nc.dram_tensor("b1r", [1, c.dh], BF16, kind="ExternalInput").ap()
    if plan.has_b2:
        b2r = nc.dram_tensor("b2r", [1, c.dh2], BF16, kind="ExternalInput").ap()
    degp = nc.dram_tensor("degp", [128, c.nt], F32, kind="ExternalInput").ap()
    if any_bias:
        sqdg = nc.dram_tensor("sqdg", [1, c.nloc], BF16, kind="ExternalInput").ap()
    idxd, idsd, cntd = [], [], []
    for i, cpb in ((1, cpb1), (2, cpb2)):
        idxd.append(nc.dram_tensor(f"idx16_{i}", [128, c.nt * cpb * 8], I16,
                                   kind="ExternalInput").ap())
        idsd.append(nc.dram_tensor(f"idsf_{i}", [128, c.nt * cpb], BF16,
                                   kind="ExternalInput").ap())
        cntd.append(nc.dram_tensor(f"cnts32_{i}", [128, c.nwin * c.nt], I32,
                                   kind="ExternalInput").ap())
    out_h = nc.dram_tensor("out_h", [c.nloc, c.dh2], F32, kind="ExternalOutput").ap()
    out_ls = nc.dram_tensor("out_ls", [c.nloc, c.dh2], F32, kind="ExternalOutput").ap()

    t1_loc = nc.dram_tensor("t1_loc", [c.nloc, c.dh], BF16, kind="Internal").ap()
    t1_full = nc.dram_tensor(
        "t1_full", [c.trows, c.dh], BF16, kind="Internal", addr_space="Shared"
    ).ap()
    t2_loc = nc.dram_tensor("t2_loc", [c.nloc, c.dt2], BF16, kind="Internal").ap()
    t2_full = nc.dram_tensor(
        "t2_full", [c.trows, c.dt2], BF16, kind="Internal", addr_space="Shared"
    ).ap()

    self_gq = [0]

    def next_q():
        q = self_gq[0]
        self_gq[0] = (q + 1) % 4
        return q

    with ExitStack() as st:
        cpool = st.enter_context(tc.tile_pool(name="consts", bufs=1))
        gp = st.enter_context(tc.tile_pool(name="gp", bufs=12))
        sp = st.enter_context(tc.tile_pool(name="sp", bufs=3))
        pp = st.enter_context(tc.tile_pool(name="pp", bufs=4))
        ppsum = st.enter_context(tc.tile_pool(name="ppsum", bufs=3, space="PSUM"))
        p0 = st.enter_context(tc.tile_pool(name="p0", bufs=3))
        p0ps = st.enter_context(tc.tile_pool(name="p0ps", bufs=2, space="PSUM"))
        p0psT = st.enter_context(tc.tile_pool(name="p0psT", bufs=1, space="PSUM"))

        # ---- constants ----
        identb = cpool.tile([128, 128], BF16)
        make_identity(nc, identb)
        w1sb = cpool.tile([128, c.kt, c.dh], BF16)
        nc.sync.dma_start(w1sb, w1.rearrange("(o p) f -> p o f", p=128))
        w2sb = cpool.tile([128, c.dh2], BF16)
        nc.sync.dma_start(w2sb, w2)
        if plan.has_b1:
            b1sb = cpool.tile([1, c.dh], BF16)
            nc.sync.dma_start(b1sb, b1r)
        if plan.has_b2:
            b2sb = cpool.tile([1, c.dh2], BF16)
            nc.sync.dma_start(b2sb, b2r)
        if any_bias:
            sqsb = cpool.tile([1, c.nloc], BF16)
            nc.sync.dma_start(sqsb, sqdg)
        dinv = cpool.tile([128, c.nt], F32)
        nc.sync.dma_start(dinv, degp)
        nc.scalar.activation(dinv, dinv, AF.Sqrt)
        nc.vector.reciprocal(dinv, dinv)
        iota = cpool.tile([128, c.max_piece, 128], BF16)
        nc.gpsimd.iota(iota, pattern=[[0, c.max_piece], [1, 128]], base=0,
                       channel_multiplier=0,
                       allow_small_or_imprecise_dtypes=True)
        cntsb = [cpool.tile([128, c.nwin * c.nt], I32, name=f"cntsb{i}")
                 for i in range(2)]
        nc.scalar.dma_start(cntsb[0], cntd[0])
        nc.scalar.dma_start(cntsb[1], cntd[1])
        cregs = [
            nc.alloc_register(mybir.EngineType.Pool, f"gcnt{w}")
            for w in range(c.nwin)
        ]
        # SBUF-resident local table shards (self-loop rows)
        sdall1 = cpool.tile([128, c.nt, c.dh], BF16)
        sdall2 = cpool.tile([128, c.nt, c.dh2], BF16)
        # log_softmax deferred state
        sm_all = cpool.tile([128, c.nt, c.dh2], F32)
        se_all = cpool.tile([128, c.nt], F32)
        ln_all = cpool.tile([128, c.nt], F32)

        # zero the gather buffers once: reg-trimmed gathers leave padding
        # slots unwritten, and stale NaN-pattern garbage would poison the
        # 0*x one-hot matmul; afterwards stale data is old finite rows
        mp = max(cpb1, cpb2)
        for _ in range(12):
            gz = gp.tile([128, mp, 128], BF16, tag="gt")
            nc.vector.memset(gz, 0.0)

        def gather_piece(t_loc, t_full, pieces, p):
            lb, gb = _piece_bounds(c, pieces)
            psz = int(lb[p + 1] - lb[p])
            nc.gpsimd.collective_compute(
                "AllGather", ALU.bypass, replica_groups=RG,
                ins=[t_loc[ds(int(lb[p]), psz), :].opt()],
                outs=[t_full[ds(int(gb[p]), c.cores * psz), :].opt()],
            )

        # ---- block-major edge aggregation ----
        def pre_w0(table_full, quotas, idx16, cnts, dt):
            # dispatch the first OB blocks' window-0 gathers ahead of the
            # in-order AllGather instruction that would block the queue
            cpb = sum(quotas)
            q0 = quotas[0]
            sit = sp.tile([128, OB * cpb * 8], I16, tag="sit")
            nc.scalar.dma_start(sit, idx16[:, : OB * cpb * 8])
            pre_g = {}
            for b in range(OB):
                g = gp.tile([128, mp, 128], BF16, tag="gt")
                nc.gpsimd.reg_load(
                    cregs, cnts[0:1, b * c.nwin : (b + 1) * c.nwin]
                )
                nc.gpsimd.dma_gather(
                    g[:, :q0, :],
                    table_full[ds(c.wbases[0], c.wsize), :],
                    sit[:, b * cpb * 8 : (b * cpb + q0) * 8],
                    num_idxs=q0 * 128, num_idxs_reg=cregs[0],
                    elem_size=dt,
                    single_packet=False, queue_num=next_q(),
                )
                pre_g[b] = g
            return pre_g

        def edge_phase(table_full, quotas, idx16, idsf, cnts, d, dt, sdall,
                       brow, post_block, hooks, pre_g):
            cpb = sum(quotas)
            offw = [0]
            for qv in quotas:
                offw.append(offw[-1] + qv)
            for ob in range(0, c.nt, OB):
                sit = sp.tile([128, OB * cpb * 8], I16, tag="sit")
                nc.scalar.dma_start(
                    sit, idx16[:, ob * cpb * 8 : (ob + OB) * cpb * 8]
                )
                sid = sp.tile([128, OB * cpb], BF16, tag="sid")
                nc.scalar.dma_start(
                    sid, idsf[:, ob * cpb : (ob + OB) * cpb]
                )
                for bi in range(OB):
                    b = ob + bi
                    pre = b in pre_g
                    g = pre_g.pop(b) if pre else \
                        gp.tile([128, mp, 128], BF16, tag="gt")
                    nc.gpsimd.reg_load(
                        cregs, cnts[0:1, b * c.nwin : (b + 1) * c.nwin]
                    )
                    for w, qv in enumerate(quotas):
                        if pre and w == 0:
                            continue
                        nc.gpsimd.dma_gather(
                            g[:, offw[w] : offw[w + 1], :],
                            table_full[ds(c.wbases[w], c.wsize), :],
                            sit[:, (bi * cpb + offw[w]) * 8
                                : (bi * cpb + offw[w + 1]) * 8],
                            num_idxs=qv * 128, num_idxs_reg=cregs[w],
                            elem_size=dt,
                            single_packet=False, queue_num=next_q(),
                        )
                    stt = pp.tile([128, cpb, 128], BF16, tag="stt")
                    nc.vector.tensor_tensor(
                        stt, iota[:, :cpb, :],
                        sid[:, bi * cpb : (bi + 1) * cpb, None].to_broadcast(
                            (128, cpb, 128)
                        ),
                        ALU.is_equal,
                    )
                    ps = ppsum.tile([128, d], F32, tag="ps")
                    if brow is not None:
                        nc.tensor.matmul(ps, lhsT=sqsb[:, ts(b, 128)],
                                         rhs=brow, start=True, stop=False)
                    nc.tensor.matmul(ps, lhsT=identb, rhs=sdall[:, b, :],
                                     start=(brow is None), stop=False)
                    for j in range(cpb):
                        nc.tensor.matmul(
                            ps, lhsT=stt[:, j, :], rhs=g[:, j, :d],
                            start=False, stop=(j == cpb - 1),
                        )
                    post_block(b, ps)
                    if b in hooks:
                        hooks[b]()

        # ---- phase 0: T1 = dinv * (x @ W1), write local table shard ----
        xv = x_sh.rearrange("(j p) n -> p j n", p=128)
        for t in range(c.nt):
            xt = p0.tile([128, c.kt, 128], BF16, tag="xt")
            (nc.sync if t % 2 else nc.scalar).dma_start(
                xt, xv[:, :, ts(t, 128)])
            hps = p0ps.tile([128, c.dh], F32, tag="hps")
            for j in range(c.kt):
                nc.tensor.matmul(
                    hps, lhsT=xt[:, j, :], rhs=w1sb[:, j, :],
                    start=(j == 0), stop=(j == c.kt - 1),
                )
            nc.scalar.activation(sdall1[:, t, :], hps, AF.Copy,
                                 scale=dinv[:, t : t + 1])
            nc.sync.dma_start(t1_loc[ts(t, 128), :], sdall1[:, t, :])
            if t == PIECES1[0] - 1:
                gather_piece(t1_loc, t1_full, PIECES1, 0)
        pre_g1 = {}
        gather_piece(t1_loc, t1_full, PIECES1, 1)

        # ---- layer 1: aggregate, then per block build the layer-2 table ----
        def post1(b, ps):
            g1 = p0.tile([128, c.dh], BF16, tag="g1")
            nc.scalar.activation(g1, ps, AF.Relu, scale=dinv[:, b : b + 1])
            g1b = p0.tile([128, c.dh], BF16, tag="g1b")
            nc.scalar.activation(g1b, g1, AF.Copy, scale=dinv[:, b : b + 1])
            tps = p0psT.tile([128, 128], BF16, tag="tps")
            nc.tensor.transpose(tps, g1b, identb)
            gT = p0.tile([128, 128], BF16, tag="gT")
            nc.vector.tensor_copy(gT, tps)
            h2ps = p0ps.tile([128, c.dh2], F32, tag="h2ps")
            nc.tensor.matmul(h2ps, lhsT=gT, rhs=w2sb, start=True, stop=True)
            nc.vector.tensor_copy(sdall2[:, b, :], h2ps)
            nc.sync.dma_start(t2_loc[ts(b, 128), : c.dh2], sdall2[:, b, :])

        # t2 AllGather pieces launch from inside the layer-1 block loop so
        # all but the small tail piece overlap the remaining layer-1 work
        pb = np.cumsum(PIECES2)
        hooks1 = {
            int(pb[0]) - 1: lambda: gather_piece(t2_loc, t2_full, PIECES2, 0),
            int(pb[1]) - 1: lambda: gather_piece(t2_loc, t2_full, PIECES2, 1),
        }
        edge_phase(t1_full, plan.quotas1, idxd[0], idsd[0], cntsb[0],
                   c.dh, c.dh, sdall1, b1sb if plan.has_b1 else None,
                   post1, hooks1, pre_g1)
        pre_g2 = {}
        gather_piece(t2_loc, t2_full, PIECES2, 2)

        # ---- layer 2: aggregate, bias + softmax statistics per block ----
        ohv = out_h.rearrange("(t p) f -> p t f", p=128)
        olv = out_ls.rearrange("(t p) f -> p t f", p=128)

        def post2(b, ps):
            oh = p0.tile([128, c.dh2], F32, tag="oh")
            nc.scalar.activation(oh, ps, AF.Copy, scale=dinv[:, b : b + 1])
            nc.sync.dma_start(ohv[:, b, :], oh)
            mx = p0.tile([128, 1], F32, tag="mx")
            nc.vector.tensor_reduce(mx, oh, mybir.AxisListType.X, ALU.max)
            nc.vector.tensor_scalar_sub(sm_all[:, b, :], oh, mx)
            e1 = p0.tile([128, c.dh2], F32, tag="e1")
            nc.scalar.activation(e1, sm_all[:, b, :], AF.Exp,
                                 accum_out=se_all[:, b : b + 1])

        edge_phase(t2_full, plan.quotas2, idxd[1], idsd[1], cntsb[1],
                   c.dh2, c.dt2, sdall2, b2sb if plan.has_b2 else None,
                   post2, {}, pre_g2)

        # ---- deferred log_softmax epilogue: one Ln, one subtract, one DMA ----
        nc.scalar.activation(ln_all, se_all, AF.Ln)
        nc.vector.tensor_tensor(
            sm_all, sm_all,
            ln_all[:, :, None].to_broadcast((128, c.nt, c.dh2)),
            ALU.subtract,
        )
        nc.sync.dma_start(olv, sm_all)


# ----------------------------------------------------------------------------
# Host entry point
# ----------------------------------------------------------------------------

_CACHE = {}


def _get_compiled(cfg: Cfg, plan: Plan):
    key = (cfg, plan)
    if key not in _CACHE:
        nc = bacc.Bacc(
            "TRN2", target_bir_lowering=False, debug=False,
            num_devices=cfg.cores, num_swdge_queues=4,
        )
        with tile.TileContext(nc) as tc:
            build(nc, tc, cfg, plan)
        nc.compile()
        _CACHE[key] = nc
    return _CACHE[key]


def make_in_maps(cfg: Cfg, plan, x, W1, b1, W2, b2, deg_pt, sqd, pack1, pack2):
    import ml_dtypes

    c = cfg
    x = np.asarray(x, np.float32)
    w2p = np.asarray(W2, np.float32)[:, : c.dh2].astype(ml_dtypes.bfloat16)
    w1c = np.ascontiguousarray(
        np.asarray(W1, np.float32).astype(ml_dtypes.bfloat16)
    )
    idx16_1, ids_1, cnts_1 = pack1
    idx16_2, ids_2, cnts_2 = pack2

    in_maps = []
    for ci in range(c.cores):
        xs = np.zeros((c.din, c.nloc), ml_dtypes.bfloat16)
        xs[:, : c.nsh] = (
            x[ci * c.nsh : (ci + 1) * c.nsh].astype(ml_dtypes.bfloat16).T
        )
        m = {
            "x_sh": np.ascontiguousarray(xs),
            "w1": w1c,
            "w2": np.ascontiguousarray(w2p),
            "degp": np.ascontiguousarray(deg_pt[ci]),
            "idx16_1": np.ascontiguousarray(idx16_1[ci]),
            "idsf_1": np.ascontiguousarray(ids_1[ci].astype(ml_dtypes.bfloat16)),
            "cnts32_1": np.ascontiguousarray(
                np.tile(cnts_1[ci][None, :], (128, 1))),
            "idx16_2": np.ascontiguousarray(idx16_2[ci]),
            "idsf_2": np.ascontiguousarray(ids_2[ci].astype(ml_dtypes.bfloat16)),
            "cnts32_2": np.ascontiguousarray(
                np.tile(cnts_2[ci][None, :], (128, 1))),
        }
        if plan.has_b1:
            m["b1r"] = np.ascontiguousarray(
                np.asarray(b1, np.float32)[None, :].astype(ml_dtypes.bfloat16))
        if plan.has_b2:
            m["b2r"] = np.ascontiguousarray(
                np.asarray(b2, np.float32)[None, : c.dh2].astype(
                    ml_dtypes.bfloat16))
        if plan.has_b1 or plan.has_b2:
            m["sqdg"] = np.ascontiguousarray(sqd[ci].astype(ml_dtypes.bfloat16))
        in_maps.append(m)
    return in_maps


def _ensure_ntff_hook():
    """Install the axon NTFF profile hook if the image's antenv lacks it."""
    import types

    try:
        from antenv.axon_hooks import get_axon_ntff_profile_hook  # noqa: F401
        return
    except ImportError:
        pass
    import antenv

    m = types.ModuleType("antenv.axon_hooks")
    m._hook = None
    m.set_axon_ntff_profile_hook = lambda h: setattr(m, "_hook", h)
    m.get_axon_ntff_profile_hook = lambda: m._hook
    sys.modules["antenv.axon_hooks"] = m
    antenv.axon_hooks = m
    try:
        from trn_agent_boot.trn_boot import _ntff_profile_via_ctypes

        h = _ntff_profile_via_ctypes("/opt/axon/libaxon_pjrt.so")
        if h is not None:
            m._hook = h
    except Exception as e:
        print(f"ntff hook install failed: {e}")

    from concourse import bass_utils as bu

    bu.upload_artifacts = lambda tmpdir: tmpdir


def run(cfg: Cfg, inputs: dict, trace: bool = False):
    if trace:
        _ensure_ntff_hook()
    deg_pt, sqd, pack1, pack2, plan = preprocess(
        cfg, inputs["edge_index"], inputs["b1"], inputs["b2"])
    nc = _get_compiled(cfg, plan)
    in_maps = make_in_maps(
        cfg, plan, inputs["x"], inputs["W1"], inputs["b1"],
        inputs["W2"], inputs["b2"], deg_pt, sqd, pack1, pack2,
    )
    res = run_bass_kernel_spmd(
        nc, in_maps, core_ids=list(range(cfg.cores)), trace=trace
    )
    c = cfg
    h = np.concatenate(
        [res.results[ci]["out_h"][: c.nsh, : c.dout] for ci in range(c.cores)], axis=0
    )
    ls = np.concatenate(
        [res.results[ci]["out_ls"][: c.nsh, : c.dout] for ci in range(c.cores)], axis=0
    )
    return (h, ls), res


def kernel(**inputs):
    (h, ls), _ = run(Cfg(), inputs)
    return h, ls
